# revision 1
# baseline (speedup 1.0000x reference)
"""Trainium2 Bass kernel for nn_MixtureOfExpertsLayer (moe_routing).

Sharding: token-data-parallel across 8 NeuronCores (1024 tokens/core),
weights replicated.  All 4 experts are computed densely per core and the
top-2 softmax gating is applied as a per-token scalar during the final
down-projection PSUM eviction (scalar_tensor_tensor accumulate), so no
gather/scatter is needed.

Layouts: activations are feature-major [128p, feature_chunk, token] so
mid-chain matmuls use the weight chunk as the stationary operand; the
final matmul of each expert flips roles (activation chunk stationary) to
produce token-major [token 128p, H] partials accumulated into `acc`.

Dtypes: float32r (tf32-like, ~1e-3 precision, full PE rate at N=512) for
expert compute; plain fp32 for the router so the top-2 selection matches
the fp32 reference.  LN stats via ones-matmul; [1,N] -> [128,N]
broadcasts via K=1 ones matmul into PSUM.
"""
import numpy as np

import concourse.bass as bass
import concourse.mybir as mybir
import concourse.tile as tile
from concourse import bacc
from concourse.alu_op_type import AluOpType
from concourse.bass_utils import run_bass_kernel_spmd

F32 = mybir.dt.float32
F32R = mybir.dt.float32r
ACT = mybir.ActivationFunctionType
AX = mybir.AxisListType
OP = AluOpType

N_CORES = 8
B, S, H, I, E = 4, 2048, 1024, 4096, 4
TOK = (B * S) // N_CORES  # tokens per core
P = 128

WEIGHT_NAMES = [
    "router_w", "router_b", "load_balancer",
    "sw_w1", "sw_w3", "sw_w2",
    "me_eq_w", "me_eq_b", "me_wv", "me_bv", "me_wo", "me_bo",
    "me_c1w", "me_c1b", "me_c2w", "me_c2b",
    "ce_syn_w", "ce_syn_b", "ce_wv", "ce_bv", "ce_wo", "ce_bo",
    "ce_n1g", "ce_n1b", "ce_f1w", "ce_f1b", "ce_f2w", "ce_f2b",
    "ce_n2g", "ce_n2b", "ce_gen_w", "ce_gen_b",
    "ml_w1", "ml_b1", "ml_w2", "ml_b2",
]


def build_moe(h=H, i_dim=I, tok=TOK):
    KC = h // P              # contraction chunks for H
    KC2 = (2 * h) // P
    TT = tok // P            # token tiles
    NTH = tok // 512         # 512-token slabs
    IB = i_dim // 512        # 512-row blocks of I
    HB = h // 512

    nc = bacc.Bacc("TRN2", target_bir_lowering=False, debug=False)

    F32R_INPUTS = {
        "sw_w1", "sw_w3", "sw_w2", "me_eq_w", "me_wv", "me_wo", "me_c1w",
        "me_c2w", "ce_syn_w", "ce_wv", "ce_wo", "ce_f1w", "ce_f2w",
        "ce_gen_w", "ml_w1", "ml_w2", "me_c2b", "ce_gen_b", "ml_b2",
    }

    def din(name, shape):
        d = F32R if name in F32R_INPUTS else F32
        return nc.dram_tensor(name, shape, d, kind="ExternalInput")

    xt = din("xt", [h, tok])
    dt = {}
    shapes = {
        "router_w": [h, E], "router_b": [E], "load_balancer": [E],
        "sw_w1": [h, i_dim], "sw_w3": [h, i_dim], "sw_w2": [i_dim, h],
        "me_eq_w": [h, h], "me_eq_b": [h], "me_wv": [h, h], "me_bv": [h],
        "me_wo": [h, h], "me_bo": [h], "me_c1w": [h, 2 * h], "me_c1b": [2 * h],
        "me_c2w": [2 * h, h], "me_c2b": [h],
        "ce_syn_w": [h, h], "ce_syn_b": [h], "ce_wv": [h, h], "ce_bv": [h],
        "ce_wo": [h, h], "ce_bo": [h], "ce_n1g": [h], "ce_n1b": [h],
        "ce_f1w": [h, 2 * h], "ce_f1b": [2 * h], "ce_f2w": [2 * h, h],
        "ce_f2b": [h], "ce_n2g": [h], "ce_n2b": [h],
        "ce_gen_w": [h, h], "ce_gen_b": [h],
        "ml_w1": [h, i_dim], "ml_b1": [i_dim], "ml_w2": [i_dim, h],
        "ml_b2": [h],
    }
    for n in WEIGHT_NAMES:
        dt[n] = din(n, shapes[n])
    out = nc.dram_tensor("out", [tok, h], F32, kind="ExternalOutput")

    def wap(w):  # [rows, cols] -> [p, row_chunk, cols]
        return w.ap().rearrange("(kc p) m -> p kc m", p=P)

    def bap(b):  # [dim] -> [p, chunk]
        return b.ap().rearrange("(mc p) -> p mc", p=P)

    cast_rr = [0]

    with tile.TileContext(nc) as tc:
        with (
            tc.tile_pool(name="const", bufs=1) as cpool,
            tc.tile_pool(name="persist", bufs=1) as ppool,
            tc.tile_pool(name="big", bufs=3) as bigp,
            tc.tile_pool(name="blk", bufs=2) as blkp,
            tc.tile_pool(name="wst", bufs=4) as wpool,
            tc.tile_pool(name="lns", bufs=1) as lnsp,
            tc.tile_pool(name="tmp", bufs=2) as tmpp,
            tc.tile_pool(name="ps", bufs=4, space=bass.MemorySpace.PSUM) as psp,
            tc.tile_pool(name="pss", bufs=2, space=bass.MemorySpace.PSUM) as pssp,
            tc.tile_pool(name="psb", bufs=2, space=bass.MemorySpace.PSUM) as psbp,
        ):
            def cast(dst, src):
                """fp32 -> fp32r cast, alternating DVE / ACT."""
                cast_rr[0] ^= 1
                if cast_rr[0]:
                    nc.vector.tensor_copy(dst, src)
                else:
                    nc.scalar.activation(dst, src, ACT.Copy)

            # ---- constants ---------------------------------------------
            ones_cf = cpool.tile([P, 1], F32, tag="ones_cf")
            nc.vector.memset(ones_cf[:], 1.0)
            ones_c = cpool.tile([P, 1], F32R, tag="ones_c")
            nc.vector.tensor_copy(ones_c[:], ones_cf[:])
            ones_rf = cpool.tile([1, P], F32, tag="ones_rf")
            nc.vector.memset(ones_rf[:], 1.0)
            ones_r = cpool.tile([1, P], F32R, tag="ones_r")
            nc.vector.tensor_copy(ones_r[:], ones_rf[:])

            def const_bias(name, mc):
                t = cpool.tile([P, mc], F32, tag=name + "_cb")
                nc.sync.dma_start(t[:], bap(dt[name]))
                return t

            def const_row_r(name, n):
                tr = cpool.tile([1, n], F32R, tag=name + "_rr")
                nc.sync.dma_start(tr[:], dt[name].ap().unsqueeze(0))
                return tr


            rb_f = cpool.tile([1, E], F32, tag="rb_f")
            nc.sync.dma_start(rb_f[:], dt["router_b"].ap().unsqueeze(0))
            lb_f = cpool.tile([1, E], F32, tag="lb_f")
            nc.sync.dma_start(lb_f[:], dt["load_balancer"].ap().unsqueeze(0))
            rblb = cpool.tile([1, E], F32, tag="rblb")
            nc.vector.tensor_tensor(rblb[:], rb_f[:], lb_f[:], OP.add)
            rw_sb = cpool.tile([P, KC, E], F32, tag="rw_sb")
            nc.sync.dma_start(rw_sb[:], wap(dt["router_w"]))

            # ---- persistent state --------------------------------------
            xr = ppool.tile([P, KC, tok], F32R, tag="xr")
            acc = ppool.tile([P, TT, h], F32, tag="acc")
            wgate = ppool.tile([P, TT, E], F32, tag="wgate")

            # ---- router + gating + x cast per 512-token slab -----------
            for sh in range(NTH):
                xf = bigp.tile([P, KC, 512], F32, tag="big")
                nc.sync.dma_start(xf[:], wap(xt)[:, :, sh * 512:(sh + 1) * 512])
                for tl in range(4):
                    t = sh * 4 + tl
                    lps = psp.tile([P, E], F32, tag="mm")
                    for kc in range(KC):
                        nc.tensor.matmul(lps[:], xf[:, kc, tl * P:(tl + 1) * P],
                                         rw_sb[:, kc, :],
                                         start=(kc == 0), stop=False)
                    nc.tensor.matmul(lps[:], ones_rf[:], rblb[:],
                                     start=False, stop=True)
                    m1 = tmpp.tile([P, 1], F32, tag="g1")
                    nc.vector.tensor_reduce(m1[:], lps[:], AX.X, OP.max)
                    ind1 = tmpp.tile([P, E], F32, tag="g2")
                    nc.vector.tensor_scalar(ind1[:], lps[:], m1[:], -1e30,
                                            OP.is_ge, OP.mult)
                    lm = tmpp.tile([P, E], F32, tag="g3")
                    nc.vector.tensor_tensor(lm[:], lps[:], ind1[:], OP.add)
                    m2 = tmpp.tile([P, 1], F32, tag="g4")
                    nc.vector.tensor_reduce(m2[:], lm[:], AX.X, OP.max)
                    nm1 = tmpp.tile([P, 1], F32, tag="g5")
                    nc.vector.tensor_scalar(nm1[:], m1[:], -1.0, None, OP.mult)
                    d = tmpp.tile([P, E], F32, tag="g6")
                    nc.vector.tensor_scalar(d[:], lps[:], nm1[:], None, OP.add)
                    ed = tmpp.tile([P, E], F32, tag="g7")
                    nc.scalar.activation(ed[:], d[:], ACT.Exp)
                    em = tmpp.tile([P, 1], F32, tag="g8")
                    nc.scalar.activation(em[:], m2[:], ACT.Exp, bias=nm1[:])
                    z = tmpp.tile([P, 1], F32, tag="g9")
                    nc.vector.tensor_scalar(z[:], em[:], 1.0, None, OP.add)
                    rz = tmpp.tile([P, 1], F32, tag="g10")
                    nc.vector.reciprocal(rz[:], z[:])
                    ind2 = tmpp.tile([P, E], F32, tag="g11")
                    nc.vector.tensor_scalar(ind2[:], lps[:], m2[:], None,
                                            OP.is_ge)
                    nc.vector.scalar_tensor_tensor(wgate[:, t, :], ed[:], rz[:],
                                                   ind2[:], OP.mult, OP.mult)
                cast(xr[:, :, sh * 512:(sh + 1) * 512], xf[:])

            eq_b_t = const_bias("me_eq_b", KC)
            bv_t = const_bias("me_bv", KC)
            bo_t = const_bias("me_bo", KC)
            c1b_t = const_bias("me_c1b", KC2)
            syn_b_t = const_bias("ce_syn_b", KC)
            cbv_t = const_bias("ce_bv", KC)
            cbo_t = const_bias("ce_bo", KC)
            f1b_t = const_bias("ce_f1b", KC2)
            f2b_t = const_bias("ce_f2b", KC)
            ml_b1_t = const_bias("ml_b1", i_dim // P)
            n1g_t = const_bias("ce_n1g", KC)
            n1b_t = const_bias("ce_n1b", KC)
            n2g_t = const_bias("ce_n2g", KC)
            n2b_t = const_bias("ce_n2b", KC)
            c2b_row = const_row_r("me_c2b", h)
            gen_b_row = const_row_r("ce_gen_b", h)
            ml_b2_row = const_row_r("ml_b2", h)

            # ---- helpers -----------------------------------------------
            def fm(dst, w_name, mc_out, src, src_off, act, bias_t, bias_col0=0,
                   w_col0=0):
                """dst[:, mc, :512] = act(W[:, cols].T @ src + b).

                Streams W in 256-column blocks; contraction over KC chunks
                of 128; 512 tokens starting at src_off."""
                w_all = wap(dt[w_name])
                for m0 in range(0, mc_out, 2):
                    wr = wpool.tile([P, KC, 256], F32R, tag="w")
                    nc.sync.dma_start(
                        wr[:],
                        w_all[:, :, w_col0 + m0 * P:w_col0 + (m0 + 2) * P])
                    for ml in range(2):
                        mc = m0 + ml
                        ps = psp.tile([P, 512], F32, tag="mm")
                        for kc in range(KC):
                            nc.tensor.matmul(
                                ps[:], wr[:, kc, ml * P:(ml + 1) * P],
                                src[:, kc, src_off:src_off + 512],
                                start=(kc == 0), stop=(kc == KC - 1))
                        if bias_t is None:
                            nc.scalar.activation(dst[:, mc, :], ps[:], act)
                        else:
                            b_sl = bias_t[:, bias_col0 + mc:bias_col0 + mc + 1]
                            f = (ACT.Identity if act == ACT.Copy else act)
                            nc.scalar.activation(dst[:, mc, :], ps[:], f,
                                                 bias=b_sl)

            def tm(w_name, rb0, kcb, src, gate_col, tok_off, init,
                   bias_row=None):
                """acc[:, tt, :] (+)= wgate[:,:,gate_col] * (src.T @ W_rows
                [+ bias]).  src is [P, kcb, 512] feature-major; W rows
                rb0*128 .. (rb0+kcb)*128 stream in [P, 4, 512] blocks."""
                w_all = wap(dt[w_name])
                nkb = (kcb + 3) // 4
                for hh in range(HB):
                    wrs = []
                    for kb in range(nkb):
                        kw = min(4, kcb - kb * 4)
                        wr = wpool.tile([P, 4, 512], F32R, tag="w")
                        nc.sync.dma_start(
                            wr[:, :kw, :],
                            w_all[:, rb0 + kb * 4:rb0 + kb * 4 + kw,
                                  hh * 512:(hh + 1) * 512])
                        wrs.append((wr, kw))
                    for tl in range(4):
                        tt = (tok_off // P) + tl
                        ps = psp.tile([P, 512], F32, tag="mm")
                        for kb, (wr, kw) in enumerate(wrs):
                            for kc in range(kw):
                                last = (kb == nkb - 1 and kc == kw - 1)
                                nc.tensor.matmul(
                                    ps[:],
                                    src[:, kb * 4 + kc, tl * P:(tl + 1) * P],
                                    wr[:, kc, :],
                                    start=(kb == 0 and kc == 0),
                                    stop=(last and bias_row is None))
                        if bias_row is not None:
                            nc.tensor.matmul(
                                ps[:], ones_r[:],
                                bias_row[0:1, hh * 512:(hh + 1) * 512],
                                start=False, stop=True)
                        a_sl = acc[:, tt, hh * 512:(hh + 1) * 512]
                        g_sl = wgate[:, tt, gate_col:gate_col + 1]
                        if init:
                            nc.vector.tensor_scalar(a_sl, ps[:], g_sl, None,
                                                    OP.mult)
                        else:
                            nc.vector.scalar_tensor_tensor(
                                a_sl, ps[:], g_sl, a_sl, OP.mult, OP.add)

            def layer_norm(dst, src, g_t, b_t):
                """dst = LN(src)*g + b over the feature dim (cross-chunk)."""
                ssum = pssp.tile([1, 512], F32, tag="st")
                for kc in range(KC):
                    nc.tensor.matmul(ssum[:], ones_c[:], src[:, kc, :],
                                     start=(kc == 0), stop=(kc == KC - 1))
                ssq = pssp.tile([1, 512], F32, tag="st")
                for half in range(KC // 4):
                    sq = blkp.tile([P, 4, 512], F32R, tag="blk")
                    nc.vector.tensor_tensor(
                        sq[:], src[:, half * 4:half * 4 + 4, :],
                        src[:, half * 4:half * 4 + 4, :], OP.mult)
                    for kc in range(4):
                        nc.tensor.matmul(ssq[:], ones_c[:], sq[:, kc, :],
                                         start=(half == 0 and kc == 0),
                                         stop=(half == KC // 4 - 1 and kc == 3))
                mu = lnsp.tile([1, 512], F32R, tag="ln1")
                nc.vector.tensor_scalar(mu[:], ssum[:], 1.0 / h, None, OP.mult)
                msq = lnsp.tile([1, 512], F32, tag="ln2")
                nc.vector.tensor_scalar(msq[:], ssq[:], 1.0 / h, None, OP.mult)
                mu2 = lnsp.tile([1, 512], F32, tag="ln3")
                nc.vector.tensor_tensor(mu2[:], mu[:], mu[:], OP.mult)
                var = lnsp.tile([1, 512], F32, tag="ln4")
                nc.vector.scalar_tensor_tensor(var[:], msq[:], 1e-5, mu2[:],
                                               OP.add, OP.subtract)
                sdev = lnsp.tile([1, 512], F32, tag="ln5a")
                nc.scalar.activation(sdev[:], var[:], ACT.Sqrt)
                rstd_f = lnsp.tile([1, 512], F32, tag="ln5f")
                nc.vector.reciprocal(rstd_f[:], sdev[:])
                rstd = lnsp.tile([1, 512], F32R, tag="ln5")
                nc.vector.tensor_copy(rstd[:], rstd_f[:])
                mub = psbp.tile([P, 512], F32, tag="bc")
                nc.tensor.matmul(mub[:], ones_r[:], mu[:], start=True,
                                 stop=True)
                rsb = psbp.tile([P, 512], F32, tag="bc")
                nc.tensor.matmul(rsb[:], ones_r[:], rstd[:], start=True,
                                 stop=True)
                for kc in range(KC):
                    t1 = tmpp.tile([P, 512], F32, tag="lnt")
                    nc.vector.tensor_tensor(t1[:], src[:, kc, :], mub[:],
                                            OP.subtract)
                    nc.vector.tensor_tensor(t1[:], t1[:], rsb[:], OP.mult)
                    nc.vector.tensor_scalar(dst[:, kc, :], t1[:],
                                            g_t[:, kc:kc + 1],
                                            b_t[:, kc:kc + 1],
                                            OP.mult, OP.add)

            # ---- expert 0: SwiGLU --------------------------------------
            w1_all, w3_all = wap(dt["sw_w1"]), wap(dt["sw_w3"])
            for ib in range(IB):
                for th in range(NTH):
                    h1 = blkp.tile([P, 4, 512], F32R, tag="blk")
                    for m0 in (0, 2):
                        c0 = ib * 512 + m0 * P
                        war = wpool.tile([P, KC, 256], F32R, tag="w")
                        nc.sync.dma_start(war[:], w1_all[:, :, c0:c0 + 256])
                        wbr = wpool.tile([P, KC, 256], F32R, tag="w")
                        nc.sync.dma_start(wbr[:], w3_all[:, :, c0:c0 + 256])
                        for ml in range(2):
                            mc = m0 + ml
                            psa = psp.tile([P, 512], F32, tag="mm")
                            psb = psp.tile([P, 512], F32, tag="mm")
                            for kc in range(KC):
                                nc.tensor.matmul(
                                    psa[:], war[:, kc, ml * P:(ml + 1) * P],
                                    xr[:, kc, th * 512:th * 512 + 512],
                                    start=(kc == 0), stop=(kc == KC - 1))
                            for kc in range(KC):
                                nc.tensor.matmul(
                                    psb[:], wbr[:, kc, ml * P:(ml + 1) * P],
                                    xr[:, kc, th * 512:th * 512 + 512],
                                    start=(kc == 0), stop=(kc == KC - 1))
                            sa = tmpp.tile([P, 512], F32, tag="sw_a")
                            nc.scalar.activation(sa[:], psa[:], ACT.Silu)
                            # fused: h1 = silu(a) * b straight off PSUM
                            nc.vector.tensor_tensor(h1[:, mc, :], psb[:],
                                                    sa[:], OP.mult)
                    tm("sw_w2", ib * 4, 4, h1, 0, th * 512,
                       init=(ib == 0))

            # ---- expert 3: GELU MLP (blocks 4.. are LN-stall fillers) --
            def e3_block(ib):
                for th in range(NTH):
                    a_r = blkp.tile([P, 4, 512], F32R, tag="blk")
                    fm(a_r, "ml_w1", 4, xr, th * 512, ACT.Gelu, ml_b1_t,
                       bias_col0=ib * 4, w_col0=ib * 512)
                    tm("ml_w2", ib * 4, 4, a_r, 3, th * 512, init=False,
                       bias_row=ml_b2_row if ib == 0 else None)

            for ib in range(IB):
                e3_block(ib)

            # ---- expert 1: MathExpert ----------------------------------
            def e1_slab(sh):
                so = sh * 512
                eq = bigp.tile([P, KC, 512], F32R, tag="big")
                fm(eq, "me_eq_w", KC, xr, so, ACT.Copy, eq_b_t)
                v1 = bigp.tile([P, KC, 512], F32R, tag="big")
                fm(v1, "me_wv", KC, eq, 0, ACT.Copy, bv_t)
                sym = bigp.tile([P, KC, 512], F32R, tag="big")
                fm(sym, "me_wo", KC, v1, 0, ACT.Copy, bo_t)
                for cb in range(KC2 // 4):
                    c1 = blkp.tile([P, 4, 512], F32R, tag="blk")
                    fm(c1, "me_c1w", 4, sym, 0, ACT.Gelu, c1b_t,
                       bias_col0=cb * 4, w_col0=cb * 512)
                    tm("me_c2w", cb * 4, 4, c1, 1, so, init=False,
                       bias_row=c2b_row if cb == 0 else None)

            # ---- expert 2: CodeExpert ----------------------------------
            def e2_slab(sh, fill1=None, fill2=None):
                so = sh * 512
                syn = bigp.tile([P, KC, 512], F32R, tag="big")
                fm(syn, "ce_syn_w", KC, xr, so, ACT.Copy, syn_b_t)
                v = bigp.tile([P, KC, 512], F32R, tag="big")
                fm(v, "ce_wv", KC, syn, 0, ACT.Copy, cbv_t)
                at = bigp.tile([P, KC, 512], F32R, tag="big")
                fm(at, "ce_wo", KC, v, 0, ACT.Copy, cbo_t)
                for kc in range(KC):
                    nc.vector.tensor_tensor(syn[:, kc, :], syn[:, kc, :],
                                            at[:, kc, :], OP.add)
                if fill1 is not None:
                    e3_block(fill1)
                h2 = bigp.tile([P, KC, 512], F32R, tag="big")
                layer_norm(h2, syn, n1g_t, n1b_t)
                ffa = bigp.tile([P, KC, 512], F32R, tag="big")
                for kc in range(KC):
                    nc.scalar.activation(ffa[:, kc, :], h2[:, kc, :], ACT.Copy)
                w2_all = wap(dt["ce_f2w"])
                for fb in range(KC2 // 4):
                    f1 = blkp.tile([P, 4, 512], F32R, tag="blk")
                    fm(f1, "ce_f1w", 4, h2, 0, ACT.Relu, f1b_t,
                       bias_col0=fb * 4, w_col0=fb * 512)
                    for half in range(HB):
                        wr = wpool.tile([P, 4, 512], F32R, tag="w")
                        nc.sync.dma_start(
                            wr[:],
                            w2_all[:, fb * 4:fb * 4 + 4,
                                   half * 512:(half + 1) * 512])
                        for ml in range(4):
                            mc = half * 4 + ml
                            ps = psp.tile([P, 512], F32, tag="mm")
                            for kc in range(4):
                                nc.tensor.matmul(
                                    ps[:], wr[:, kc, ml * P:(ml + 1) * P],
                                    f1[:, kc, :],
                                    start=(kc == 0), stop=(kc == 3))
                            if fb == 0:
                                nc.vector.scalar_tensor_tensor(
                                    ffa[:, mc, :], ps[:],
                                    f2b_t[:, mc:mc + 1], ffa[:, mc, :],
                                    OP.add, OP.add)
                            else:
                                nc.vector.tensor_tensor(
                                    ffa[:, mc, :], ps[:], ffa[:, mc, :],
                                    OP.add)
                if fill2 is not None:
                    e3_block(fill2)
                h2b = bigp.tile([P, KC, 512], F32R, tag="big")
                layer_norm(h2b, ffa, n2g_t, n2b_t)
                tm("ce_gen_w", 0, KC, h2b, 2, so, init=False,
                   bias_row=gen_b_row)
                # acc token-tiles for this slab are final: store them now
                nc.sync.dma_start(
                    out.ap().rearrange("(tt p) m -> p tt m", p=P)[
                        :, sh * 4:(sh + 1) * 4, :],
                    acc[:, sh * 4:(sh + 1) * 4, :])


            for sh in range(NTH):
                e1_slab(sh)
                e2_slab(sh)

    nc.compile()
    return nc


_PROGRAM = None


def _get_program():
    global _PROGRAM
    if _PROGRAM is None:
        _PROGRAM = build_moe()
    return _PROGRAM


def run_cores(nc, in_maps, trace=False, trace_cores=None):
    if trace:
        _install_ntff_shim()
    return run_bass_kernel_spmd(nc, in_maps, core_ids=list(range(len(in_maps))),
                                trace=trace, trace_cores=trace_cores)


def make_in_maps(inputs):
    base = {n: np.ascontiguousarray(np.asarray(inputs[n], np.float32))
            for n in WEIGHT_NAMES}
    x = np.asarray(inputs["x"], np.float32).reshape(-1, H)
    in_maps = []
    for c in range(N_CORES):
        xt_c = np.ascontiguousarray(x[c * TOK:(c + 1) * TOK].T)
        in_maps.append({**base, "xt": xt_c})
    return in_maps


def kernel(**inputs):
    nc = _get_program()
    res = run_cores(nc, make_in_maps(inputs))
    outs = [res.results[c]["out"] for c in range(N_CORES)]
    x = np.asarray(inputs["x"])
    return np.concatenate(outs, 0).reshape(x.shape).astype(np.float32)


# ---- NTFF profiling shim (axon) — used by test.py only ----------------
def _install_ntff_shim():
    import contextlib
    import ctypes
    import sys
    import types

    if "antenv.axon_hooks" in sys.modules:
        return
    lib = ctypes.CDLL("/opt/axon/libaxon_pjrt.so")
    if not hasattr(lib, "axon_start_nrt_profile"):
        return
    lib.axon_start_nrt_profile.argtypes = [ctypes.POINTER(ctypes.c_int64),
                                           ctypes.c_size_t]
    lib.axon_start_nrt_profile.restype = ctypes.c_int64
    lib.axon_stop_nrt_profile.argtypes = [ctypes.c_char_p]
    lib.axon_stop_nrt_profile.restype = ctypes.c_int64

    @contextlib.contextmanager
    def _hook(output_dir, device_ids):
        import jax
        jax.devices()
        if device_ids:
            ids = (ctypes.c_int64 * len(device_ids))(*device_ids)
            rc = lib.axon_start_nrt_profile(ids, len(device_ids))
        else:
            rc = lib.axon_start_nrt_profile(None, 0)
        if rc != 0:
            raise RuntimeError(f"axon_start_nrt_profile rc={rc}")
        try:
            yield
        finally:
            n = lib.axon_stop_nrt_profile(str(output_dir).encode())
            print(f"profile: {n} file(s) written to {output_dir}",
                  file=sys.stderr)

    import antenv
    mod = types.ModuleType("antenv.axon_hooks")
    mod.get_axon_ntff_profile_hook = lambda: _hook
    mod.set_axon_ntff_profile_hook = lambda hk: None
    sys.modules["antenv.axon_hooks"] = mod
    antenv.axon_hooks = mod



# revision 9
# speedup vs baseline: 2.5956x; 2.5956x over previous
"""Trainium2 Bass kernel for nn_MixtureOfExpertsLayer (moe_routing).

Sparse dispatch: top-2 routing is computed on the host (the router is a
tiny [8192,1024]@[1024,4] GEMM); tokens are gathered per expert and
sharded across the 8 cores so each core runs a fixed 512-token slab
through each of the 4 experts — half the dense FLOPs.  The linear
chains inside experts 1/2 are pre-folded on the host
(eq_w@wv@wo and syn_w@(I+wv@wo)), removing another ~11% of matmul work.

Device compute is bf16 (PSUM accumulates fp32).  Weights are pre-packed
on the host into the exact [p, kc, 256-col] tile layout the tensor
engine wants, so every DMA is a fully contiguous 0.5-2MB block.
Activations stay feature-major [128p, chunk, tok]; every matmul has a
512-token moving dim (full PE rate).  Expert outputs come back
feature-major [H, 512] fp32; the host applies the top-2 softmax gates
and scatter-adds into the final output.  Tokens beyond the
4096-per-expert device capacity (a few dozen when routing is balanced)
are computed on the host in fp64.
"""
import math

import numpy as np
import ml_dtypes

import concourse.bass as bass
import concourse.mybir as mybir
import concourse.tile as tile
from concourse import bacc
from concourse.alu_op_type import AluOpType
from concourse.bass_utils import run_bass_kernel_spmd

F32 = mybir.dt.float32
BF16 = mybir.dt.bfloat16
ACT = mybir.ActivationFunctionType
AX = mybir.AxisListType
OP = AluOpType
BF = ml_dtypes.bfloat16

N_CORES = 8
B, S, H, I, E = 4, 2048, 1024, 4096, 4
P = 128
T = 512                   # tokens per expert per core
CAP = N_CORES * T         # device capacity per expert
KC = H // P               # 8
KI = I // P               # 32
K2 = (2 * H) // P         # 16

# packed weight dram tensors: name -> (n_256col_blocks, contraction_chunks)
PACKED_W = {
    "w1p": (I // 256, KC), "w3p": (I // 256, KC), "m1p": (I // 256, KC),
    "w2p": (H // 256, KI), "m2p": (H // 256, KI),
    "c1p": (2 * H // 256, KC), "f1p": (2 * H // 256, KC),
    "c2p": (H // 256, K2), "f2p": (H // 256, K2),
    "a1p": (H // 256, KC), "a2p": (H // 256, KC), "genp": (H // 256, KC),
}
# bias dram tensors: name -> n_chunks (each [P, n])
BIASES = {
    "a1b": KC, "c1b": K2, "c2b": KC,
    "a2b": KC, "f1b": K2, "f2b": KC,
    "n1g": KC, "n1b": KC, "n2g": KC, "n2b": KC, "genb": KC,
    "m1b": KI, "m2b": KC,
}


def build_moe_sparse():
    nc = bacc.Bacc("TRN2", target_bir_lowering=False, debug=False)

    xg = [nc.dram_tensor(f"xg{e}", [P, KC, T], BF16, kind="ExternalInput")
          for e in range(E)]
    wd = {n: nc.dram_tensor(n, [nb, P, kcc, 256], BF16, kind="ExternalInput")
          for n, (nb, kcc) in PACKED_W.items()}
    bd = {n: nc.dram_tensor(n, [P, nch], F32, kind="ExternalInput")
          for n, nch in BIASES.items()}
    ys = [nc.dram_tensor(f"y{e}", [P, KC, T], F32, kind="ExternalOutput")
          for e in range(E)]

    with tile.TileContext(nc) as tc:
        with (
            tc.tile_pool(name="const", bufs=1) as cpool,
            tc.tile_pool(name="xg", bufs=2) as xpool,
            tc.tile_pool(name="h1", bufs=1) as hpool,
            tc.tile_pool(name="inter", bufs=1) as ipool,
            tc.tile_pool(name="ws", bufs=4) as wsp,     # KC-contraction blocks
            tc.tile_pool(name="ws2", bufs=3) as wsp2,   # K2-contraction blocks
            tc.tile_pool(name="wb", bufs=2) as wbp,     # KI-contraction blocks
            tc.tile_pool(name="yev", bufs=3) as ypool,  # output eviction
            tc.tile_pool(name="lns", bufs=1) as lnsp,
            tc.tile_pool(name="lnt", bufs=2) as lntp,
            tc.tile_pool(name="sq", bufs=2) as sqp,
            tc.tile_pool(name="ps", bufs=4, space=bass.MemorySpace.PSUM) as psp,
            tc.tile_pool(name="pss", bufs=2, space=bass.MemorySpace.PSUM) as pssp,
            tc.tile_pool(name="psb", bufs=2, space=bass.MemorySpace.PSUM) as psbp,
        ):
            # ---- constants ------------------------------------------------
            ones_cf = cpool.tile([P, 1], F32, tag="ones_cf")
            nc.vector.memset(ones_cf[:], 1.0)
            ones_c = cpool.tile([P, 1], BF16, tag="ones_c")
            nc.vector.tensor_copy(ones_c[:], ones_cf[:])
            ones_rf = cpool.tile([1, P], F32, tag="ones_rf")
            nc.vector.memset(ones_rf[:], 1.0)
            ones_r = cpool.tile([1, P], BF16, tag="ones_r")
            nc.vector.tensor_copy(ones_r[:], ones_rf[:])

            bt = {}
            for n, nch in BIASES.items():
                bt[n] = cpool.tile([P, nch], F32, tag=n, name="b_" + n)
                nc.sync.dma_start(bt[n][:], bd[n].ap())

            def load_xg(e):
                t_ = xpool.tile([P, KC, T], BF16, tag="xg", name=f"xgt{e}")
                nc.sync.dma_start(t_[:], xg[e].ap())
                return t_

            h1 = hpool.tile([P, KI, T], BF16, tag="h1")

            # ---- helpers --------------------------------------------------
            def up_proj(dst, wname, src, src_kc, act, bias, blocks=None,
                        pool=None):
                """dst[:, c, :] = act(Wc.T @ src + bias_c), streamed in
                256-col blocks.  dst chunk c = 2*b + ml."""
                pool = pool or wsp
                nb = PACKED_W[wname][0]
                for b_ in (range(nb) if blocks is None else blocks):
                    wc = pool.tile([P, src_kc, 256], BF16, tag="w")
                    nc.sync.dma_start(wc[:], wd[wname].ap()[b_])
                    for ml in range(2):
                        c = 2 * b_ + ml
                        ps = psp.tile([P, T], F32, tag="mm")
                        for kc in range(src_kc):
                            nc.tensor.matmul(
                                ps[:], wc[:, kc, ml * P:(ml + 1) * P],
                                src[:, kc, :],
                                start=(kc == 0), stop=(kc == src_kc - 1))
                        b_sl = None if bias is None else bias[:, c:c + 1]
                        nc.scalar.activation(dst[:, c, :], ps[:], act,
                                             bias=b_sl)

            def out_proj(ydram, wname, src, src_kc, bias, wpool):
                """y[:, c, :] = Wc.T @ src + bias_c -> DMA to DRAM (fp32)."""
                nb = PACKED_W[wname][0]
                for b_ in range(nb):
                    wc = wpool.tile([P, src_kc, 256], BF16, tag="w")
                    nc.sync.dma_start(wc[:], wd[wname].ap()[b_])
                    for ml in range(2):
                        c = 2 * b_ + ml
                        ps = psp.tile([P, T], F32, tag="mm")
                        for kc in range(src_kc):
                            nc.tensor.matmul(
                                ps[:], wc[:, kc, ml * P:(ml + 1) * P],
                                src[:, kc, :],
                                start=(kc == 0), stop=(kc == src_kc - 1))
                        yt = ypool.tile([P, T], F32, tag="y")
                        nc.vector.tensor_scalar(yt[:], ps[:],
                                                bias[:, c:c + 1], None, OP.add)
                        nc.sync.dma_start(ydram.ap()[:, c, :], yt[:])

            def ln_stats(src, tag):
                """Mean/rstd rows (bf16 [1,T]) of feature-major src."""
                ssum = pssp.tile([1, T], F32, tag="st")
                for kc in range(KC):
                    nc.tensor.matmul(ssum[:], ones_c[:], src[:, kc, :],
                                     start=(kc == 0), stop=(kc == KC - 1))
                ssq = pssp.tile([1, T], F32, tag="st")
                for kc in range(KC):
                    sqc = sqp.tile([P, T], BF16, tag="sq")
                    nc.vector.tensor_tensor(sqc[:], src[:, kc, :],
                                            src[:, kc, :], OP.mult)
                    nc.tensor.matmul(ssq[:], ones_c[:], sqc[:],
                                     start=(kc == 0), stop=(kc == KC - 1))
                mu = lnsp.tile([1, T], F32, tag="mu")
                nc.vector.tensor_scalar(mu[:], ssum[:], 1.0 / H, None, OP.mult)
                msq = lnsp.tile([1, T], F32, tag="ms")
                nc.vector.tensor_scalar(msq[:], ssq[:], 1.0 / H, None, OP.mult)
                mu_b = lnsp.tile([1, T], BF16, tag=tag + "mb")
                nc.vector.tensor_copy(mu_b[:], mu[:])
                mu2 = lnsp.tile([1, T], F32, tag="m2")
                nc.vector.tensor_tensor(mu2[:], mu[:], mu[:], OP.mult)
                var = lnsp.tile([1, T], F32, tag="va")
                nc.vector.scalar_tensor_tensor(var[:], msq[:], 1e-5, mu2[:],
                                               OP.add, OP.subtract)
                sdev = lnsp.tile([1, T], F32, tag="sd")
                nc.scalar.activation(sdev[:], var[:], ACT.Sqrt)
                rstd_f = lnsp.tile([1, T], F32, tag="rf")
                nc.vector.reciprocal(rstd_f[:], sdev[:])
                rs_b = lnsp.tile([1, T], BF16, tag=tag + "rb")
                nc.vector.tensor_copy(rs_b[:], rstd_f[:])
                return mu_b, rs_b

            def ln_apply(dst, src, mu_b, rs_b, g_t, b_t):
                """dst = (src - mu) * rstd * g + b  (bf16 out)."""
                mub = psbp.tile([P, T], F32, tag="bc")
                nc.tensor.matmul(mub[:], ones_r[:], mu_b[:], start=True,
                                 stop=True)
                rsb = psbp.tile([P, T], F32, tag="bc")
                nc.tensor.matmul(rsb[:], ones_r[:], rs_b[:], start=True,
                                 stop=True)
                for kc in range(KC):
                    t1_ = lntp.tile([P, T], F32, tag="lnt")
                    nc.vector.tensor_tensor(t1_[:], src[:, kc, :], mub[:],
                                            OP.subtract)
                    nc.vector.tensor_tensor(t1_[:], t1_[:], rsb[:], OP.mult)
                    nc.vector.tensor_scalar(dst[:, kc, :], t1_[:],
                                            g_t[:, kc:kc + 1],
                                            b_t[:, kc:kc + 1],
                                            OP.mult, OP.add)

            # ---- expert 0: SwiGLU ----------------------------------------
            xt0 = load_xg(0)
            xt2 = load_xg(2)
            for b_ in range(I // 256):
                wa = wsp.tile([P, KC, 256], BF16, tag="w")
                nc.sync.dma_start(wa[:], wd["w1p"].ap()[b_])
                wb = wsp.tile([P, KC, 256], BF16, tag="w")
                nc.sync.dma_start(wb[:], wd["w3p"].ap()[b_])
                for ml in range(2):
                    c = 2 * b_ + ml
                    psa = psp.tile([P, T], F32, tag="mm")
                    psb = psp.tile([P, T], F32, tag="mm")
                    for kc in range(KC):
                        nc.tensor.matmul(psa[:], wa[:, kc, ml * P:(ml + 1) * P],
                                         xt0[:, kc, :],
                                         start=(kc == 0), stop=(kc == KC - 1))
                    for kc in range(KC):
                        nc.tensor.matmul(psb[:], wb[:, kc, ml * P:(ml + 1) * P],
                                         xt0[:, kc, :],
                                         start=(kc == 0), stop=(kc == KC - 1))
                    sa = ypool.tile([P, T], F32, tag="sa")
                    nc.scalar.activation(sa[:], psa[:], ACT.Silu)
                    nc.vector.tensor_tensor(h1[:, c, :], psb[:], sa[:],
                                            OP.mult)
            # sw_w2 has no bias in the reference: use a zero bias tile
            zb = cpool.tile([P, KC], F32, tag="zb")
            nc.vector.memset(zb[:], 0.0)
            out_proj(ys[0], "w2p", h1, KI, zb, wbp)

            # ---- expert 2 (part 1): folded front + LN1 stats -------------
            t2 = ipool.tile([P, KC, T], BF16, tag="tA", name="t2")
            up_proj(t2, "a2p", xt2, KC, ACT.Identity, bt["a2b"])
            mu1, rs1 = ln_stats(t2, "l1")

            # ---- expert 1 (filler for LN1 latency) -----------------------
            xt1 = load_xg(1)
            t1 = ipool.tile([P, KC, T], BF16, tag="tB", name="t1")
            up_proj(t1, "a1p", xt1, KC, ACT.Identity, bt["a1b"])
            h2 = ipool.tile([P, KC, T], BF16, tag="tC", name="h2")
            ln_apply(h2, t2, mu1, rs1, bt["n1g"], bt["n1b"])
            g1 = ipool.tile([P, K2, T], BF16, tag="tD", name="g1")
            up_proj(g1, "c1p", t1, KC, ACT.Gelu, bt["c1b"])
            out_proj(ys[1], "c2p", g1, K2, bt["c2b"], wsp2)

            # ---- expert 2 (part 2): FF + residual + LN2 stats ------------
            g2 = ipool.tile([P, K2, T], BF16, tag="tD", name="g2")
            up_proj(g2, "f1p", h2, KC, ACT.Relu, bt["f1b"])
            ffa = ipool.tile([P, KC, T], BF16, tag="tB", name="ffa")
            nb_f2 = PACKED_W["f2p"][0]
            for b_ in range(nb_f2):
                wc = wsp2.tile([P, K2, 256], BF16, tag="w")
                nc.sync.dma_start(wc[:], wd["f2p"].ap()[b_])
                for ml in range(2):
                    c = 2 * b_ + ml
                    ps = psp.tile([P, T], F32, tag="mm")
                    for kc in range(K2):
                        nc.tensor.matmul(ps[:], wc[:, kc, ml * P:(ml + 1) * P],
                                         g2[:, kc, :],
                                         start=(kc == 0), stop=(kc == K2 - 1))
                    # ffa = ff + f2b + h2   (residual)
                    nc.vector.scalar_tensor_tensor(
                        ffa[:, c, :], ps[:], bt["f2b"][:, c:c + 1],
                        h2[:, c, :], OP.add, OP.add)
            mu2, rs2 = ln_stats(ffa, "l2")

            # ---- expert 3 up-projection (filler for LN2 latency) ---------
            xt3 = load_xg(3)
            up_proj(h1, "m1p", xt3, KC, ACT.Gelu, bt["m1b"])

            # ---- expert 2 (part 3): LN2 apply + generator ----------------
            h2b = ipool.tile([P, KC, T], BF16, tag="tA", name="h2b")
            ln_apply(h2b, ffa, mu2, rs2, bt["n2g"], bt["n2b"])
            out_proj(ys[2], "genp", h2b, KC, bt["genb"], wsp)

            # ---- expert 3 down-projection --------------------------------
            out_proj(ys[3], "m2p", h1, KI, bt["m2b"], wbp)

    nc.compile()
    return nc


_PROGRAM = None


def _get_program():
    global _PROGRAM
    if _PROGRAM is None:
        _PROGRAM = build_moe_sparse()
    return _PROGRAM


def run_cores(nc, in_maps, trace=False, trace_cores=None):
    if trace:
        _install_ntff_shim()
    return run_bass_kernel_spmd(nc, in_maps, core_ids=list(range(len(in_maps))),
                                trace=trace, trace_cores=trace_cores)


# ---- host side ---------------------------------------------------------
def _gelu(x):
    try:
        from scipy.special import erf
        return 0.5 * x * (1.0 + erf(x / math.sqrt(2.0)))
    except ImportError:
        ve = np.vectorize(math.erf)
        return 0.5 * x * (1.0 + ve(x / math.sqrt(2.0)))


def _ln64(h, g, b, eps=1e-5):
    mu = h.mean(-1, keepdims=True)
    var = ((h - mu) ** 2).mean(-1, keepdims=True)
    return (h - mu) / np.sqrt(var + eps) * g + b


def _pack_w(w, kcc):
    """[K, M] fp64 -> [M//256, P, kcc, 256] bf16 contiguous tile blocks."""
    K, M = w.shape
    assert K == kcc * P
    r = w.reshape(kcc, P, M)
    blocks = [np.ascontiguousarray(r[:, :, b * 256:(b + 1) * 256]
                                   .transpose(1, 0, 2))
              for b in range(M // 256)]
    return np.stack(blocks, 0).astype(BF)


def _pack_b(b):
    n = b.shape[0] // P
    return np.ascontiguousarray(b.reshape(n, P).T.astype(np.float32))


def prepare(inputs):
    f64 = lambda n: np.asarray(inputs[n], np.float64)
    x = np.asarray(inputs["x"], np.float32).reshape(-1, H)

    # routing (host, fp64)
    lg = x.astype(np.float64) @ f64("router_w")
    lg += f64("router_b") + f64("load_balancer")
    sel = np.argsort(-lg, axis=1, kind="stable")[:, :2]
    ls = np.take_along_axis(lg, sel, 1)
    ew = np.exp(ls - ls.max(1, keepdims=True))
    gates = ew / ew.sum(1, keepdims=True)

    # folded weights (fp64)
    F = {}
    F["A1"] = f64("me_eq_w") @ f64("me_wv") @ f64("me_wo")
    F["a1"] = (f64("me_eq_b") @ f64("me_wv") + f64("me_bv")) @ f64("me_wo") \
        + f64("me_bo")
    W2o = f64("ce_wv") @ f64("ce_wo")
    F["A2"] = f64("ce_syn_w") + f64("ce_syn_w") @ W2o
    F["a2"] = f64("ce_syn_b") + f64("ce_syn_b") @ W2o + f64("ce_bv") \
        @ f64("ce_wo") + f64("ce_bo")

    wmap = {
        "w1p": (f64("sw_w1"), KC), "w3p": (f64("sw_w3"), KC),
        "w2p": (f64("sw_w2"), KI),
        "a1p": (F["A1"], KC), "c1p": (f64("me_c1w"), KC),
        "c2p": (f64("me_c2w"), K2),
        "a2p": (F["A2"], KC), "f1p": (f64("ce_f1w"), KC),
        "f2p": (f64("ce_f2w"), K2), "genp": (f64("ce_gen_w"), KC),
        "m1p": (f64("ml_w1"), KC), "m2p": (f64("ml_w2"), KI),
    }
    bmap = {
        "a1b": F["a1"], "c1b": f64("me_c1b"), "c2b": f64("me_c2b"),
        "a2b": F["a2"], "f1b": f64("ce_f1b"), "f2b": f64("ce_f2b"),
        "n1g": f64("ce_n1g"), "n1b": f64("ce_n1b"),
        "n2g": f64("ce_n2g"), "n2b": f64("ce_n2b"),
        "genb": f64("ce_gen_b"), "m1b": f64("ml_b1"), "m2b": f64("ml_b2"),
    }
    base = {n: _pack_w(w, kcc) for n, (w, kcc) in wmap.items()}
    base.update({n: _pack_b(b) for n, b in bmap.items()})

    meta = {"x": x, "gates": gates, "sel": sel, "F": F,
            "dev_idx": [], "dev_w": [], "ovf": []}
    in_maps = [dict(base) for _ in range(N_CORES)]
    for e in range(E):
        m = sel == e
        tok = np.nonzero(m.any(1))[0]
        we = np.where(m[:, 0][tok], gates[tok, 0], gates[tok, 1])
        dev, ovf = tok[:CAP], tok[CAP:]
        meta["dev_idx"].append(dev)
        meta["dev_w"].append(we[:len(dev)])
        meta["ovf"].append((ovf, we[len(dev):]))
        xfull = np.zeros((CAP, H), np.float32)
        xfull[:len(dev)] = x[dev]
        percore = xfull.reshape(N_CORES, T, H)
        for c in range(N_CORES):
            xc = percore[c].T.reshape(KC, P, T).transpose(1, 0, 2)
            in_maps[c][f"xg{e}"] = np.ascontiguousarray(xc).astype(BF)
    meta["in_maps"] = in_maps
    return meta


def _host_expert(e, xs, inputs, F):
    """Overflow tokens, fp64, replicating the reference formulas."""
    f64 = lambda n: np.asarray(inputs[n], np.float64)
    xs = xs.astype(np.float64)
    if e == 0:
        a = xs @ f64("sw_w1")
        g = a / (1.0 + np.exp(-a)) * (xs @ f64("sw_w3"))
        return g @ f64("sw_w2")
    if e == 1:
        t = xs @ F["A1"] + F["a1"]
        g = _gelu(t @ f64("me_c1w") + f64("me_c1b"))
        return g @ f64("me_c2w") + f64("me_c2b")
    if e == 2:
        t = xs @ F["A2"] + F["a2"]
        h2 = _ln64(t, f64("ce_n1g"), f64("ce_n1b"))
        ff = np.maximum(h2 @ f64("ce_f1w") + f64("ce_f1b"), 0.0) \
            @ f64("ce_f2w") + f64("ce_f2b")
        h2 = _ln64(h2 + ff, f64("ce_n2g"), f64("ce_n2b"))
        return h2 @ f64("ce_gen_w") + f64("ce_gen_b")
    a = _gelu(xs @ f64("ml_w1") + f64("ml_b1"))
    return a @ f64("ml_w2") + f64("ml_b2")


def combine(meta, results, inputs):
    out = np.zeros((B * S, H), np.float32)
    for e in range(E):
        ye = np.concatenate(
            [results[c][f"y{e}"].transpose(2, 1, 0).reshape(T, H)
             for c in range(N_CORES)], 0)
        dev, we = meta["dev_idx"][e], meta["dev_w"][e]
        out[dev] += (we[:, None] * ye[:len(dev)]).astype(np.float32)
        ovf, wo = meta["ovf"][e]
        if len(ovf):
            yh = _host_expert(e, meta["x"][ovf], inputs, meta["F"])
            out[ovf] += (wo[:, None] * yh).astype(np.float32)
    return out.reshape(B, S, H)


def kernel(**inputs):
    nc = _get_program()
    meta = prepare(inputs)
    res = run_cores(nc, meta["in_maps"])
    return combine(meta, [res.results[c] for c in range(N_CORES)], inputs)


# ---- NTFF profiling shim (axon) — used by test.py only ----------------
def _install_ntff_shim():
    import contextlib
    import ctypes
    import sys
    import types

    if "antenv.axon_hooks" in sys.modules:
        return
    lib = ctypes.CDLL("/opt/axon/libaxon_pjrt.so")
    if not hasattr(lib, "axon_start_nrt_profile"):
        return
    lib.axon_start_nrt_profile.argtypes = [ctypes.POINTER(ctypes.c_int64),
                                           ctypes.c_size_t]
    lib.axon_start_nrt_profile.restype = ctypes.c_int64
    lib.axon_stop_nrt_profile.argtypes = [ctypes.c_char_p]
    lib.axon_stop_nrt_profile.restype = ctypes.c_int64

    @contextlib.contextmanager
    def _hook(output_dir, device_ids):
        import jax
        jax.devices()
        if device_ids:
            ids = (ctypes.c_int64 * len(device_ids))(*device_ids)
            rc = lib.axon_start_nrt_profile(ids, len(device_ids))
        else:
            rc = lib.axon_start_nrt_profile(None, 0)
        if rc != 0:
            raise RuntimeError(f"axon_start_nrt_profile rc={rc}")
        try:
            yield
        finally:
            n = lib.axon_stop_nrt_profile(str(output_dir).encode())
            print(f"profile: {n} file(s) written to {output_dir}",
                  file=sys.stderr)

    import antenv
    mod = types.ModuleType("antenv.axon_hooks")
    mod.get_axon_ntff_profile_hook = lambda: _hook
    mod.set_axon_ntff_profile_hook = lambda hk: None
    sys.modules["antenv.axon_hooks"] = mod
    antenv.axon_hooks = mod


# revision 16
# speedup vs baseline: 2.7430x; 1.0568x over previous
"""Trainium2 Bass kernel for nn_MixtureOfExpertsLayer (moe_routing).

Sparse dispatch: top-2 routing is computed on the host (the router is a
tiny [8192,1024]@[1024,4] GEMM); tokens are gathered per expert and
sharded across the 8 cores so each core runs a fixed 512-token slab
through each of the 4 experts — half the dense FLOPs.  The linear
chains inside experts 1/2 are pre-folded on the host
(eq_w@wv@wo and syn_w@(I+wv@wo)), removing another ~11% of matmul work.

Device compute is bf16 (PSUM accumulates fp32).  Weights are pre-packed
on the host into the exact [p, kc, 256-col] tile layout the tensor
engine wants, so every DMA is a fully contiguous 0.5-2MB block.
Activations stay feature-major [128p, chunk, tok]; every matmul has a
512-token moving dim (full PE rate).  Expert outputs come back
feature-major [H, 512] fp32; the host applies the top-2 softmax gates
and scatter-adds into the final output.  Tokens beyond the
4096-per-expert device capacity (a few dozen when routing is balanced)
are computed on the host in fp64.
"""
import math

import numpy as np
import ml_dtypes

import concourse.bass as bass
import concourse.mybir as mybir
import concourse.tile as tile
from concourse import bacc
from concourse.alu_op_type import AluOpType
from concourse.bass_utils import run_bass_kernel_spmd

F32 = mybir.dt.float32
BF16 = mybir.dt.bfloat16
ACT = mybir.ActivationFunctionType
AX = mybir.AxisListType
OP = AluOpType
BF = ml_dtypes.bfloat16

N_CORES = 8
B, S, H, I, E = 4, 2048, 1024, 4096, 4
P = 128
T = 512                   # tokens per expert per core
CAP = N_CORES * T         # device capacity per expert
KC = H // P               # 8
KI = I // P               # 32
K2 = (2 * H) // P         # 16

# packed weight dram tensors: name -> (n_256col_blocks, contraction_chunks)
PACKED_W = {
    "w1p": (I // 256, KC), "w3p": (I // 256, KC), "m1p": (I // 256, KC),
    "w2p": (H // 256, KI), "m2p": (H // 256, KI),
    "c1p": (2 * H // 256, KC), "f1p": (2 * H // 256, KC),
    "c2p": (H // 256, K2), "f2p": (H // 256, K2),
    "a2p": (H // 256, KC), "genp": (H // 256, KC),
}
# biases live in one packed [P, sum] f32 tensor; name -> n_chunks
BIASES = {
    "c1b": K2, "c2b": KC,
    "a2b": KC, "f1b": K2, "f2b": KC,
    "n1g": KC, "n1b": KC, "n2g": KC, "n2b": KC, "genb": KC,
    "m1b": KI, "m2b": KC, "zb": KC,
}
BIAS_OFF = {}
_off = 0
for _n, _c in BIASES.items():
    BIAS_OFF[_n] = _off
    _off += _c
BIAS_COLS = _off


def build_moe_sparse():
    nc = bacc.Bacc("TRN2", target_bir_lowering=False, debug=False)

    xg = [nc.dram_tensor(f"xg{e}", [P, KC, T], BF16, kind="ExternalInput")
          for e in range(E)]
    wd = {n: nc.dram_tensor(n, [nb, P, kcc, 256], BF16, kind="ExternalInput")
          for n, (nb, kcc) in PACKED_W.items()}
    ball = nc.dram_tensor("ball", [P, BIAS_COLS], F32, kind="ExternalInput")
    ys = [nc.dram_tensor(f"y{e}", [P, KC, T], F32, kind="ExternalOutput")
          for e in range(E)]

    with tile.TileContext(nc) as tc:
        with (
            tc.tile_pool(name="const", bufs=1) as cpool,
            tc.tile_pool(name="xg", bufs=2) as xpool,
            tc.tile_pool(name="h1", bufs=1) as hpool,
            tc.tile_pool(name="inter", bufs=1) as ipool,
            tc.tile_pool(name="ws", bufs=4) as wsp,     # KC-contraction blocks
            tc.tile_pool(name="ws2", bufs=3) as wsp2,   # K2-contraction blocks
            tc.tile_pool(name="wb", bufs=2) as wbp,     # KI-contraction blocks
            tc.tile_pool(name="yev", bufs=3) as ypool,  # output eviction
            tc.tile_pool(name="lns", bufs=1) as lnsp,
            tc.tile_pool(name="lnt", bufs=2) as lntp,
            tc.tile_pool(name="sq", bufs=2) as sqp,
            tc.tile_pool(name="ps", bufs=4, space=bass.MemorySpace.PSUM) as psp,
            tc.tile_pool(name="pss", bufs=2, space=bass.MemorySpace.PSUM) as pssp,
            tc.tile_pool(name="psb", bufs=2, space=bass.MemorySpace.PSUM) as psbp,
        ):
            # ---- constants ------------------------------------------------
            ones_cf = cpool.tile([P, 1], F32, tag="ones_cf")
            nc.vector.memset(ones_cf[:], 1.0)
            ones_c = cpool.tile([P, 1], BF16, tag="ones_c")
            nc.vector.tensor_copy(ones_c[:], ones_cf[:])
            ones_rf = cpool.tile([1, P], F32, tag="ones_rf")
            nc.vector.memset(ones_rf[:], 1.0)
            ones_r = cpool.tile([1, P], BF16, tag="ones_r")
            nc.vector.tensor_copy(ones_r[:], ones_rf[:])

            bt_all = cpool.tile([P, BIAS_COLS], F32, tag="ball")
            bt = {n: bt_all[:, BIAS_OFF[n]:BIAS_OFF[n] + nch]
                  for n, nch in BIASES.items()}

            def load_xg(e):
                t_ = xpool.tile([P, KC, T], BF16, tag="xg", name=f"xgt{e}")
                nc.sync.dma_start(t_[:], xg[e].ap())
                return t_

            h1 = hpool.tile([P, KI, T], BF16, tag="h1")

            # ---- helpers --------------------------------------------------
            def up_proj(dst, wname, src, src_kc, act, bias, blocks=None,
                        pool=None):
                """dst[:, c, :] = act(Wc.T @ src + bias_c), streamed in
                256-col blocks.  dst chunk c = 2*b + ml."""
                pool = pool or wsp
                nb = PACKED_W[wname][0]
                for b_ in (range(nb) if blocks is None else blocks):
                    wc = pool.tile([P, src_kc, 256], BF16, tag="w")
                    nc.sync.dma_start(wc[:], wd[wname].ap()[b_])
                    for ml in range(2):
                        c = 2 * b_ + ml
                        ps = psp.tile([P, T], F32, tag="mm")
                        for kc in range(src_kc):
                            nc.tensor.matmul(
                                ps[:], wc[:, kc, ml * P:(ml + 1) * P],
                                src[:, kc, :],
                                start=(kc == 0), stop=(kc == src_kc - 1))
                        b_sl = None if bias is None else bias[:, c:c + 1]
                        nc.scalar.activation(dst[:, c, :], ps[:], act,
                                             bias=b_sl)

            def out_proj(ydram, wname, src, src_kc, bias, wpool):
                """y[:, c, :] = Wc.T @ src + bias_c -> DMA to DRAM (fp32)."""
                nb = PACKED_W[wname][0]
                for b_ in range(nb):
                    wc = wpool.tile([P, src_kc, 256], BF16, tag="w")
                    nc.sync.dma_start(wc[:], wd[wname].ap()[b_])
                    for ml in range(2):
                        c = 2 * b_ + ml
                        ps = psp.tile([P, T], F32, tag="mm")
                        for kc in range(src_kc):
                            nc.tensor.matmul(
                                ps[:], wc[:, kc, ml * P:(ml + 1) * P],
                                src[:, kc, :],
                                start=(kc == 0), stop=(kc == src_kc - 1))
                        yt = ypool.tile([P, T], F32, tag="y")
                        nc.vector.tensor_scalar(yt[:], ps[:],
                                                bias[:, c:c + 1], None, OP.add)
                        nc.sync.dma_start(ydram.ap()[:, c, :], yt[:])

            def ln_stats(src, tag):
                """Mean/rstd rows (bf16 [1,T]) of feature-major src."""
                ssum = pssp.tile([1, T], F32, tag="st")
                for kc in range(KC):
                    nc.tensor.matmul(ssum[:], ones_c[:], src[:, kc, :],
                                     start=(kc == 0), stop=(kc == KC - 1))
                ssq = pssp.tile([1, T], F32, tag="st")
                for kc in range(KC):
                    sqc = sqp.tile([P, T], BF16, tag="sq")
                    nc.vector.tensor_tensor(sqc[:], src[:, kc, :],
                                            src[:, kc, :], OP.mult)
                    nc.tensor.matmul(ssq[:], ones_c[:], sqc[:],
                                     start=(kc == 0), stop=(kc == KC - 1))
                mu = lnsp.tile([1, T], F32, tag="mu")
                nc.vector.tensor_scalar(mu[:], ssum[:], 1.0 / H, None, OP.mult)
                msq = lnsp.tile([1, T], F32, tag="ms")
                nc.vector.tensor_scalar(msq[:], ssq[:], 1.0 / H, None, OP.mult)
                mu_b = lnsp.tile([1, T], BF16, tag=tag + "mb")
                nc.vector.tensor_copy(mu_b[:], mu[:])
                mu2 = lnsp.tile([1, T], F32, tag="m2")
                nc.vector.tensor_tensor(mu2[:], mu[:], mu[:], OP.mult)
                var = lnsp.tile([1, T], F32, tag="va")
                nc.vector.scalar_tensor_tensor(var[:], msq[:], 1e-5, mu2[:],
                                               OP.add, OP.subtract)
                sdev = lnsp.tile([1, T], F32, tag="sd")
                nc.scalar.activation(sdev[:], var[:], ACT.Sqrt)
                rstd_f = lnsp.tile([1, T], F32, tag="rf")
                nc.vector.reciprocal(rstd_f[:], sdev[:])
                rs_b = lnsp.tile([1, T], BF16, tag=tag + "rb")
                nc.vector.tensor_copy(rs_b[:], rstd_f[:])
                return mu_b, rs_b

            def ln_apply(dst, src, mu_b, rs_b, g_t, b_t):
                """dst = (src - mu) * rstd * g + b  (bf16 out)."""
                mub = psbp.tile([P, T], F32, tag="bc")
                nc.tensor.matmul(mub[:], ones_r[:], mu_b[:], start=True,
                                 stop=True)
                rsb = psbp.tile([P, T], F32, tag="bc")
                nc.tensor.matmul(rsb[:], ones_r[:], rs_b[:], start=True,
                                 stop=True)
                for kc in range(KC):
                    t1_ = lntp.tile([P, T], F32, tag="lnt")
                    nc.vector.tensor_tensor(t1_[:], src[:, kc, :], mub[:],
                                            OP.subtract)
                    nc.vector.tensor_tensor(t1_[:], t1_[:], rsb[:], OP.mult)
                    nc.vector.tensor_scalar(dst[:, kc, :], t1_[:],
                                            g_t[:, kc:kc + 1],
                                            b_t[:, kc:kc + 1],
                                            OP.mult, OP.add)

            # ---- expert 0: SwiGLU ----------------------------------------
            xt0 = load_xg(0)
            for b_ in range(I // 256):
                wa = wsp.tile([P, KC, 256], BF16, tag="w")
                nc.sync.dma_start(wa[:], wd["w1p"].ap()[b_])
                wb = wsp.tile([P, KC, 256], BF16, tag="w")
                nc.sync.dma_start(wb[:], wd["w3p"].ap()[b_])
                if b_ == 1:
                    # defer non-critical loads so startup DMA bandwidth goes
                    # to xg0 + the first SwiGLU weight blocks
                    nc.sync.dma_start(bt_all[:], ball.ap())
                    xt2 = load_xg(2)
                for ml in range(2):
                    c = 2 * b_ + ml
                    psa = psp.tile([P, T], F32, tag="mm")
                    psb = psp.tile([P, T], F32, tag="mm")
                    for kc in range(KC):
                        nc.tensor.matmul(psa[:], wa[:, kc, ml * P:(ml + 1) * P],
                                         xt0[:, kc, :],
                                         start=(kc == 0), stop=(kc == KC - 1))
                    for kc in range(KC):
                        nc.tensor.matmul(psb[:], wb[:, kc, ml * P:(ml + 1) * P],
                                         xt0[:, kc, :],
                                         start=(kc == 0), stop=(kc == KC - 1))
                    sa = ypool.tile([P, T], F32, tag="sa")
                    nc.scalar.activation(sa[:], psa[:], ACT.Silu)
                    nc.vector.tensor_tensor(h1[:, c, :], psb[:], sa[:],
                                            OP.mult)
            out_proj(ys[0], "w2p", h1, KI, bt["zb"], wbp)

            # ---- expert 2 (part 1): folded front + LN1 stats -------------
            t2 = ipool.tile([P, KC, T], BF16, tag="tA", name="t2")
            up_proj(t2, "a2p", xt2, KC, ACT.Identity, bt["a2b"])
            mu1, rs1 = ln_stats(t2, "l1")

            # ---- expert 1 (filler for LN1 latency); c1p holds A1@C1 ------
            xt1 = load_xg(1)
            g1 = ipool.tile([P, K2, T], BF16, tag="tD", name="g1")
            up_proj(g1, "c1p", xt1, KC, ACT.Gelu, bt["c1b"])
            h2 = ipool.tile([P, KC, T], BF16, tag="tC", name="h2")
            ln_apply(h2, t2, mu1, rs1, bt["n1g"], bt["n1b"])
            out_proj(ys[1], "c2p", g1, K2, bt["c2b"], wsp2)

            # ---- expert 2 (part 2): FF + residual + LN2 stats ------------
            g2 = ipool.tile([P, K2, T], BF16, tag="tD", name="g2")
            up_proj(g2, "f1p", h2, KC, ACT.Relu, bt["f1b"])
            ffa = ipool.tile([P, KC, T], BF16, tag="tB", name="ffa")
            nb_f2 = PACKED_W["f2p"][0]
            for b_ in range(nb_f2):
                wc = wsp2.tile([P, K2, 256], BF16, tag="w")
                nc.sync.dma_start(wc[:], wd["f2p"].ap()[b_])
                for ml in range(2):
                    c = 2 * b_ + ml
                    ps = psp.tile([P, T], F32, tag="mm")
                    for kc in range(K2):
                        nc.tensor.matmul(ps[:], wc[:, kc, ml * P:(ml + 1) * P],
                                         g2[:, kc, :],
                                         start=(kc == 0), stop=(kc == K2 - 1))
                    # ffa = ff + f2b + h2   (residual)
                    nc.vector.scalar_tensor_tensor(
                        ffa[:, c, :], ps[:], bt["f2b"][:, c:c + 1],
                        h2[:, c, :], OP.add, OP.add)
            mu2, rs2 = ln_stats(ffa, "l2")

            # ---- expert 3 up-proj first half (filler for LN2 latency) ----
            xt3 = load_xg(3)
            up_proj(h1, "m1p", xt3, KC, ACT.Gelu, bt["m1b"],
                    blocks=range(0, 8))

            # ---- expert 2 (part 3): LN2 apply + generator ----------------
            h2b = ipool.tile([P, KC, T], BF16, tag="tA", name="h2b")
            ln_apply(h2b, ffa, mu2, rs2, bt["n2g"], bt["n2b"])
            out_proj(ys[2], "genp", h2b, KC, bt["genb"], wsp)

            # ---- expert 3 up-proj second half + down-projection ----------
            up_proj(h1, "m1p", xt3, KC, ACT.Gelu, bt["m1b"],
                    blocks=range(8, 16))
            out_proj(ys[3], "m2p", h1, KI, bt["m2b"], wbp)

    nc.compile()
    return nc


_PROGRAM = None


def _get_program():
    global _PROGRAM
    if _PROGRAM is None:
        _PROGRAM = build_moe_sparse()
    return _PROGRAM


def run_cores(nc, in_maps, trace=False, trace_cores=None):
    if trace:
        _install_ntff_shim()
    return run_bass_kernel_spmd(nc, in_maps, core_ids=list(range(len(in_maps))),
                                trace=trace, trace_cores=trace_cores)


# ---- host side ---------------------------------------------------------
def _gelu(x):
    try:
        from scipy.special import erf
        return 0.5 * x * (1.0 + erf(x / math.sqrt(2.0)))
    except ImportError:
        ve = np.vectorize(math.erf)
        return 0.5 * x * (1.0 + ve(x / math.sqrt(2.0)))


def _ln64(h, g, b, eps=1e-5):
    mu = h.mean(-1, keepdims=True)
    var = ((h - mu) ** 2).mean(-1, keepdims=True)
    return (h - mu) / np.sqrt(var + eps) * g + b


def _pack_w(w, kcc):
    """[K, M] fp64 -> [M//256, P, kcc, 256] bf16 contiguous tile blocks."""
    K, M = w.shape
    assert K == kcc * P
    r = w.reshape(kcc, P, M)
    blocks = [np.ascontiguousarray(r[:, :, b * 256:(b + 1) * 256]
                                   .transpose(1, 0, 2))
              for b in range(M // 256)]
    return np.stack(blocks, 0).astype(BF)


def _pack_b(b):
    n = b.shape[0] // P
    return np.ascontiguousarray(b.reshape(n, P).T.astype(np.float32))


def prepare(inputs):
    f64 = lambda n: np.asarray(inputs[n], np.float64)
    x = np.asarray(inputs["x"], np.float32).reshape(-1, H)

    # routing (host, fp64)
    lg = x.astype(np.float64) @ f64("router_w")
    lg += f64("router_b") + f64("load_balancer")
    sel = np.argsort(-lg, axis=1, kind="stable")[:, :2]
    ls = np.take_along_axis(lg, sel, 1)
    ew = np.exp(ls - ls.max(1, keepdims=True))
    gates = ew / ew.sum(1, keepdims=True)

    # folded weights (fp64)
    F = {}
    F["A1"] = f64("me_eq_w") @ f64("me_wv") @ f64("me_wo")
    F["a1"] = (f64("me_eq_b") @ f64("me_wv") + f64("me_bv")) @ f64("me_wo") \
        + f64("me_bo")
    W2o = f64("ce_wv") @ f64("ce_wo")
    F["A2"] = f64("ce_syn_w") + f64("ce_syn_w") @ W2o
    F["a2"] = f64("ce_syn_b") + f64("ce_syn_b") @ W2o + f64("ce_bv") \
        @ f64("ce_wo") + f64("ce_bo")

    wmap = {
        "w1p": (f64("sw_w1"), KC), "w3p": (f64("sw_w3"), KC),
        "w2p": (f64("sw_w2"), KI),
        "c1p": (F["A1"] @ f64("me_c1w"), KC),
        "c2p": (f64("me_c2w"), K2),
        "a2p": (F["A2"], KC), "f1p": (f64("ce_f1w"), KC),
        "f2p": (f64("ce_f2w"), K2), "genp": (f64("ce_gen_w"), KC),
        "m1p": (f64("ml_w1"), KC), "m2p": (f64("ml_w2"), KI),
    }
    bmap = {
        "c1b": F["a1"] @ f64("me_c1w") + f64("me_c1b"),
        "c2b": f64("me_c2b"),
        "a2b": F["a2"],
        "f1b": f64("ce_f1b"), "f2b": f64("ce_f2b"),
        "n1g": f64("ce_n1g"), "n1b": f64("ce_n1b"),
        "n2g": f64("ce_n2g"), "n2b": f64("ce_n2b"),
        "genb": f64("ce_gen_b"), "m1b": f64("ml_b1"), "m2b": f64("ml_b2"),
        "zb": np.zeros(H),
    }
    base = {n: _pack_w(w, kcc) for n, (w, kcc) in wmap.items()}
    base["ball"] = np.concatenate([_pack_b(bmap[n]) for n in BIASES], 1)

    meta = {"x": x, "gates": gates, "sel": sel, "F": F,
            "dev_idx": [], "dev_w": [], "ovf": []}
    in_maps = [dict(base) for _ in range(N_CORES)]
    for e in range(E):
        m = sel == e
        tok = np.nonzero(m.any(1))[0]
        we = np.where(m[:, 0][tok], gates[tok, 0], gates[tok, 1])
        dev, ovf = tok[:CAP], tok[CAP:]
        meta["dev_idx"].append(dev)
        meta["dev_w"].append(we[:len(dev)])
        meta["ovf"].append((ovf, we[len(dev):]))
        xfull = np.zeros((CAP, H), np.float32)
        xfull[:len(dev)] = x[dev]
        percore = xfull.reshape(N_CORES, T, H)
        for c in range(N_CORES):
            xc = percore[c].T.reshape(KC, P, T).transpose(1, 0, 2)
            in_maps[c][f"xg{e}"] = np.ascontiguousarray(xc).astype(BF)
    meta["in_maps"] = in_maps
    return meta


def _host_expert(e, xs, inputs, F):
    """Overflow tokens, fp64, replicating the reference formulas."""
    f64 = lambda n: np.asarray(inputs[n], np.float64)
    xs = xs.astype(np.float64)
    if e == 0:
        a = xs @ f64("sw_w1")
        g = a / (1.0 + np.exp(-a)) * (xs @ f64("sw_w3"))
        return g @ f64("sw_w2")
    if e == 1:
        t = xs @ F["A1"] + F["a1"]
        g = _gelu(t @ f64("me_c1w") + f64("me_c1b"))
        return g @ f64("me_c2w") + f64("me_c2b")
    if e == 2:
        t = xs @ F["A2"] + F["a2"]
        h2 = _ln64(t, f64("ce_n1g"), f64("ce_n1b"))
        ff = np.maximum(h2 @ f64("ce_f1w") + f64("ce_f1b"), 0.0) \
            @ f64("ce_f2w") + f64("ce_f2b")
        h2 = _ln64(h2 + ff, f64("ce_n2g"), f64("ce_n2b"))
        return h2 @ f64("ce_gen_w") + f64("ce_gen_b")
    a = _gelu(xs @ f64("ml_w1") + f64("ml_b1"))
    return a @ f64("ml_w2") + f64("ml_b2")


def combine(meta, results, inputs):
    out = np.zeros((B * S, H), np.float32)
    for e in range(E):
        ye = np.concatenate(
            [results[c][f"y{e}"].transpose(2, 1, 0).reshape(T, H)
             for c in range(N_CORES)], 0)
        dev, we = meta["dev_idx"][e], meta["dev_w"][e]
        out[dev] += (we[:, None] * ye[:len(dev)]).astype(np.float32)
        ovf, wo = meta["ovf"][e]
        if len(ovf):
            yh = _host_expert(e, meta["x"][ovf], inputs, meta["F"])
            out[ovf] += (wo[:, None] * yh).astype(np.float32)
    return out.reshape(B, S, H)


def kernel(**inputs):
    nc = _get_program()
    meta = prepare(inputs)
    res = run_cores(nc, meta["in_maps"])
    return combine(meta, [res.results[c] for c in range(N_CORES)], inputs)


# ---- NTFF profiling shim (axon) — used by test.py only ----------------
def _install_ntff_shim():
    import contextlib
    import ctypes
    import sys
    import types

    if "antenv.axon_hooks" in sys.modules:
        return
    lib = ctypes.CDLL("/opt/axon/libaxon_pjrt.so")
    if not hasattr(lib, "axon_start_nrt_profile"):
        return
    lib.axon_start_nrt_profile.argtypes = [ctypes.POINTER(ctypes.c_int64),
                                           ctypes.c_size_t]
    lib.axon_start_nrt_profile.restype = ctypes.c_int64
    lib.axon_stop_nrt_profile.argtypes = [ctypes.c_char_p]
    lib.axon_stop_nrt_profile.restype = ctypes.c_int64

    @contextlib.contextmanager
    def _hook(output_dir, device_ids):
        import jax
        jax.devices()
        if device_ids:
            ids = (ctypes.c_int64 * len(device_ids))(*device_ids)
            rc = lib.axon_start_nrt_profile(ids, len(device_ids))
        else:
            rc = lib.axon_start_nrt_profile(None, 0)
        if rc != 0:
            raise RuntimeError(f"axon_start_nrt_profile rc={rc}")
        try:
            yield
        finally:
            n = lib.axon_stop_nrt_profile(str(output_dir).encode())
            print(f"profile: {n} file(s) written to {output_dir}",
                  file=sys.stderr)

    import antenv
    mod = types.ModuleType("antenv.axon_hooks")
    mod.get_axon_ntff_profile_hook = lambda: _hook
    mod.set_axon_ntff_profile_hook = lambda hk: None
    sys.modules["antenv.axon_hooks"] = mod
    antenv.axon_hooks = mod


# revision 20
# speedup vs baseline: 2.7652x; 1.0081x over previous
"""Trainium2 Bass kernel for nn_MixtureOfExpertsLayer (moe_routing).

Sparse dispatch: top-2 routing is computed on the host (the router is a
tiny [8192,1024]@[1024,4] GEMM); tokens are gathered per expert and
sharded across the 8 cores so each core runs a fixed 512-token slab
through each of the 4 experts — half the dense FLOPs.  The linear
chains inside experts 1/2 are pre-folded on the host
(eq_w@wv@wo and syn_w@(I+wv@wo)), removing another ~11% of matmul work.

Device compute is bf16 (PSUM accumulates fp32).  Weights are pre-packed
on the host into the exact [p, kc, 256-col] tile layout the tensor
engine wants, so every DMA is a fully contiguous 0.5-2MB block.
Activations stay feature-major [128p, chunk, tok]; every matmul has a
512-token moving dim (full PE rate).  Expert outputs come back
feature-major [H, 512] fp32; the host applies the top-2 softmax gates
and scatter-adds into the final output.  Tokens beyond the
4096-per-expert device capacity (a few dozen when routing is balanced)
are computed on the host in fp64.
"""
import math

import numpy as np
import ml_dtypes

import concourse.bass as bass
import concourse.mybir as mybir
import concourse.tile as tile
from concourse import bacc
from concourse.alu_op_type import AluOpType
from concourse.bass_utils import run_bass_kernel_spmd

F32 = mybir.dt.float32
BF16 = mybir.dt.bfloat16
ACT = mybir.ActivationFunctionType
AX = mybir.AxisListType
OP = AluOpType
BF = ml_dtypes.bfloat16

N_CORES = 8
B, S, H, I, E = 4, 2048, 1024, 4096, 4
P = 128
T = 512                   # tokens per expert per core
CAP = N_CORES * T         # device capacity per expert
KC = H // P               # 8
KI = I // P               # 32
K2 = (2 * H) // P         # 16

# packed weight dram tensors: name -> (n_256col_blocks, contraction_chunks)
PACKED_W = {
    "w1p": (I // 256, KC), "w3p": (I // 256, KC), "m1p": (I // 256, KC),
    "w2p": (H // 256, KI), "m2p": (H // 256, KI),
    "c1p": (2 * H // 256, KC), "f1p": (2 * H // 256, KC),
    "c2p": (H // 256, K2), "f2p": (H // 256, K2),
    "a2p": (H // 256, KC), "genp": (H // 256, KC),
}
# biases live in one packed [P, sum] f32 tensor; name -> n_chunks
BIASES = {
    "c1b": K2, "c2b": KC,
    "a2b": KC, "f1b": K2, "f2b": KC,
    "n1g": KC, "n1b": KC, "n2g": KC, "n2b": KC, "genb": KC,
    "m1b": KI, "m2b": KC, "zb": KC,
}
BIAS_OFF = {}
_off = 0
for _n, _c in BIASES.items():
    BIAS_OFF[_n] = _off
    _off += _c
BIAS_COLS = _off


def build_moe_sparse():
    nc = bacc.Bacc("TRN2", target_bir_lowering=False, debug=False)

    xg = [nc.dram_tensor(f"xg{e}", [P, KC, T], BF16, kind="ExternalInput")
          for e in range(E)]
    wd = {n: nc.dram_tensor(n, [nb, P, kcc, 256], BF16, kind="ExternalInput")
          for n, (nb, kcc) in PACKED_W.items()}
    ball = nc.dram_tensor("ball", [P, BIAS_COLS], F32, kind="ExternalInput")
    ys = [nc.dram_tensor(f"y{e}", [P, KC, T], F32, kind="ExternalOutput")
          for e in range(E)]

    with tile.TileContext(nc) as tc:
        with (
            tc.tile_pool(name="const", bufs=1) as cpool,
            tc.tile_pool(name="xg", bufs=2) as xpool,
            tc.tile_pool(name="h1", bufs=1) as hpool,
            tc.tile_pool(name="inter", bufs=1) as ipool,
            tc.tile_pool(name="ws", bufs=4) as wsp,     # KC-contraction blocks
            tc.tile_pool(name="ws2", bufs=3) as wsp2,   # K2-contraction blocks
            tc.tile_pool(name="wb", bufs=2) as wbp,     # KI-contraction blocks
            tc.tile_pool(name="yev", bufs=3) as ypool,  # output eviction
            tc.tile_pool(name="lns", bufs=1) as lnsp,
            tc.tile_pool(name="lnt", bufs=2) as lntp,
            tc.tile_pool(name="sq", bufs=2) as sqp,
            tc.tile_pool(name="ps", bufs=4, space=bass.MemorySpace.PSUM) as psp,
            tc.tile_pool(name="pss", bufs=2, space=bass.MemorySpace.PSUM) as pssp,
            tc.tile_pool(name="psb", bufs=2, space=bass.MemorySpace.PSUM) as psbp,
        ):
            # ---- constants ------------------------------------------------
            ones_cf = cpool.tile([P, 1], F32, tag="ones_cf")
            nc.vector.memset(ones_cf[:], 1.0)
            ones_c = cpool.tile([P, 1], BF16, tag="ones_c")
            nc.vector.tensor_copy(ones_c[:], ones_cf[:])
            ones_rf = cpool.tile([1, P], F32, tag="ones_rf")
            nc.vector.memset(ones_rf[:], 1.0)
            ones_r = cpool.tile([1, P], BF16, tag="ones_r")
            nc.vector.tensor_copy(ones_r[:], ones_rf[:])

            bt_all = cpool.tile([P, BIAS_COLS], F32, tag="ball")
            bt = {n: bt_all[:, BIAS_OFF[n]:BIAS_OFF[n] + nch]
                  for n, nch in BIASES.items()}

            def load_xg(e, split=1):
                t_ = xpool.tile([P, KC, T], BF16, tag="xg", name=f"xgt{e}")
                step = KC // split
                for i in range(split):
                    sl = slice(i * step, (i + 1) * step)
                    nc.sync.dma_start(t_[:, sl, :], xg[e].ap()[:, sl, :])
                return t_

            h1 = hpool.tile([P, KI, T], BF16, tag="h1")

            # ---- helpers --------------------------------------------------
            def up_proj(dst, wname, src, src_kc, act, bias, blocks=None,
                        pool=None):
                """dst[:, c, :] = act(Wc.T @ src + bias_c), streamed in
                256-col blocks.  dst chunk c = 2*b + ml."""
                pool = pool or wsp
                nb = PACKED_W[wname][0]
                for b_ in (range(nb) if blocks is None else blocks):
                    wc = pool.tile([P, src_kc, 256], BF16, tag="w")
                    nc.sync.dma_start(wc[:], wd[wname].ap()[b_])
                    for ml in range(2):
                        c = 2 * b_ + ml
                        ps = psp.tile([P, T], F32, tag="mm")
                        for kc in range(src_kc):
                            nc.tensor.matmul(
                                ps[:], wc[:, kc, ml * P:(ml + 1) * P],
                                src[:, kc, :],
                                start=(kc == 0), stop=(kc == src_kc - 1))
                        b_sl = None if bias is None else bias[:, c:c + 1]
                        nc.scalar.activation(dst[:, c, :], ps[:], act,
                                             bias=b_sl)

            def out_proj(ydram, wname, src, src_kc, bias, wpool):
                """y[:, c, :] = Wc.T @ src + bias_c -> DMA to DRAM (fp32)."""
                nb = PACKED_W[wname][0]
                for b_ in range(nb):
                    wc = wpool.tile([P, src_kc, 256], BF16, tag="w")
                    nc.sync.dma_start(wc[:], wd[wname].ap()[b_])
                    for ml in range(2):
                        c = 2 * b_ + ml
                        ps = psp.tile([P, T], F32, tag="mm")
                        for kc in range(src_kc):
                            nc.tensor.matmul(
                                ps[:], wc[:, kc, ml * P:(ml + 1) * P],
                                src[:, kc, :],
                                start=(kc == 0), stop=(kc == src_kc - 1))
                        yt = ypool.tile([P, T], F32, tag="y")
                        nc.vector.tensor_scalar(yt[:], ps[:],
                                                bias[:, c:c + 1], None, OP.add)
                        nc.sync.dma_start(ydram.ap()[:, c, :], yt[:])

            def ln_stats(src, tag):
                """Mean/rstd rows (bf16 [1,T]) of feature-major src."""
                ssum = pssp.tile([1, T], F32, tag="st")
                for kc in range(KC):
                    nc.tensor.matmul(ssum[:], ones_c[:], src[:, kc, :],
                                     start=(kc == 0), stop=(kc == KC - 1))
                ssq = pssp.tile([1, T], F32, tag="st")
                for kc in range(KC):
                    sqc = sqp.tile([P, T], BF16, tag="sq")
                    nc.vector.tensor_tensor(sqc[:], src[:, kc, :],
                                            src[:, kc, :], OP.mult)
                    nc.tensor.matmul(ssq[:], ones_c[:], sqc[:],
                                     start=(kc == 0), stop=(kc == KC - 1))
                mu = lnsp.tile([1, T], F32, tag="mu")
                nc.vector.tensor_scalar(mu[:], ssum[:], 1.0 / H, None, OP.mult)
                msq = lnsp.tile([1, T], F32, tag="ms")
                nc.vector.tensor_scalar(msq[:], ssq[:], 1.0 / H, None, OP.mult)
                mu_b = lnsp.tile([1, T], BF16, tag=tag + "mb")
                nc.vector.tensor_copy(mu_b[:], mu[:])
                mu2 = lnsp.tile([1, T], F32, tag="m2")
                nc.vector.tensor_tensor(mu2[:], mu[:], mu[:], OP.mult)
                var = lnsp.tile([1, T], F32, tag="va")
                nc.vector.scalar_tensor_tensor(var[:], msq[:], 1e-5, mu2[:],
                                               OP.add, OP.subtract)
                sdev = lnsp.tile([1, T], F32, tag="sd")
                nc.scalar.activation(sdev[:], var[:], ACT.Sqrt)
                rstd_f = lnsp.tile([1, T], F32, tag="rf")
                nc.vector.reciprocal(rstd_f[:], sdev[:])
                rs_b = lnsp.tile([1, T], BF16, tag=tag + "rb")
                nc.vector.tensor_copy(rs_b[:], rstd_f[:])
                return mu_b, rs_b

            def ln_bcast(mu_b, rs_b):
                """Broadcast [1,T] mean/rstd rows to [P,T] via K=1 matmul."""
                mub = psbp.tile([P, T], F32, tag="bc")
                nc.tensor.matmul(mub[:], ones_r[:], mu_b[:], start=True,
                                 stop=True)
                rsb = psbp.tile([P, T], F32, tag="bc")
                nc.tensor.matmul(rsb[:], ones_r[:], rs_b[:], start=True,
                                 stop=True)
                return mub, rsb

            def ln_norm(dst, src, mub, rsb, g_t, b_t):
                """dst = (src - mu) * rstd * g + b  (bf16 out, DVE only)."""
                for kc in range(KC):
                    t1_ = lntp.tile([P, T], F32, tag="lnt")
                    nc.vector.tensor_tensor(t1_[:], src[:, kc, :], mub[:],
                                            OP.subtract)
                    nc.vector.tensor_tensor(t1_[:], t1_[:], rsb[:], OP.mult)
                    nc.vector.tensor_scalar(dst[:, kc, :], t1_[:],
                                            g_t[:, kc:kc + 1],
                                            b_t[:, kc:kc + 1],
                                            OP.mult, OP.add)

            def ln_apply(dst, src, mu_b, rs_b, g_t, b_t):
                mub, rsb = ln_bcast(mu_b, rs_b)
                ln_norm(dst, src, mub, rsb, g_t, b_t)

            # ---- expert 0: SwiGLU ----------------------------------------
            xt0 = load_xg(0, split=2)
            for b_ in range(I // 256):
                wa = wsp.tile([P, KC, 256], BF16, tag="w")
                nc.sync.dma_start(wa[:], wd["w1p"].ap()[b_])
                wb = wsp.tile([P, KC, 256], BF16, tag="w")
                nc.sync.dma_start(wb[:], wd["w3p"].ap()[b_])
                if b_ == 1:
                    # defer non-critical loads so startup DMA bandwidth goes
                    # to xg0 + the first SwiGLU weight blocks
                    nc.sync.dma_start(bt_all[:], ball.ap())
                    xt2 = load_xg(2)
                for ml in range(2):
                    c = 2 * b_ + ml
                    psa = psp.tile([P, T], F32, tag="mm")
                    psb = psp.tile([P, T], F32, tag="mm")
                    for kc in range(KC):
                        nc.tensor.matmul(psa[:], wa[:, kc, ml * P:(ml + 1) * P],
                                         xt0[:, kc, :],
                                         start=(kc == 0), stop=(kc == KC - 1))
                    for kc in range(KC):
                        nc.tensor.matmul(psb[:], wb[:, kc, ml * P:(ml + 1) * P],
                                         xt0[:, kc, :],
                                         start=(kc == 0), stop=(kc == KC - 1))
                    sa = ypool.tile([P, T], F32, tag="sa")
                    nc.scalar.activation(sa[:], psa[:], ACT.Silu)
                    nc.vector.tensor_tensor(h1[:, c, :], psb[:], sa[:],
                                            OP.mult)
            out_proj(ys[0], "w2p", h1, KI, bt["zb"], wbp)

            # ---- expert 2 (part 1): folded front + LN1 stats -------------
            t2 = ipool.tile([P, KC, T], BF16, tag="tA", name="t2")
            up_proj(t2, "a2p", xt2, KC, ACT.Identity, bt["a2b"])
            mu1, rs1 = ln_stats(t2, "l1")

            # ---- expert 1 (filler for LN1 latency); c1p holds A1@C1 ------
            xt1 = load_xg(1)
            g1 = ipool.tile([P, K2, T], BF16, tag="tD", name="g1")
            up_proj(g1, "c1p", xt1, KC, ACT.Gelu, bt["c1b"])
            h2 = ipool.tile([P, KC, T], BF16, tag="tC", name="h2")
            ln_apply(h2, t2, mu1, rs1, bt["n1g"], bt["n1b"])
            out_proj(ys[1], "c2p", g1, K2, bt["c2b"], wsp2)

            # ---- expert 2 (part 2): FF + residual + LN2 stats ------------
            g2 = ipool.tile([P, K2, T], BF16, tag="tD", name="g2")
            up_proj(g2, "f1p", h2, KC, ACT.Relu, bt["f1b"])
            ffa = ipool.tile([P, KC, T], BF16, tag="tB", name="ffa")
            nb_f2 = PACKED_W["f2p"][0]
            for b_ in range(nb_f2):
                wc = wsp2.tile([P, K2, 256], BF16, tag="w")
                nc.sync.dma_start(wc[:], wd["f2p"].ap()[b_])
                for ml in range(2):
                    c = 2 * b_ + ml
                    ps = psp.tile([P, T], F32, tag="mm")
                    for kc in range(K2):
                        nc.tensor.matmul(ps[:], wc[:, kc, ml * P:(ml + 1) * P],
                                         g2[:, kc, :],
                                         start=(kc == 0), stop=(kc == K2 - 1))
                    # ffa = ff + f2b + h2   (residual)
                    nc.vector.scalar_tensor_tensor(
                        ffa[:, c, :], ps[:], bt["f2b"][:, c:c + 1],
                        h2[:, c, :], OP.add, OP.add)
            # ---- expert 3 up-proj interleaved with LN2 + generator -------
            # E3 blocks fill the PE while the DVE drains ffa evictions,
            # computes LN2 stats rows, and normalizes h2b.
            xt3 = load_xg(3)
            up_proj(h1, "m1p", xt3, KC, ACT.Gelu, bt["m1b"],
                    blocks=range(0, 2))
            mu2, rs2 = ln_stats(ffa, "l2")
            mub2, rsb2 = ln_bcast(mu2, rs2)
            up_proj(h1, "m1p", xt3, KC, ACT.Gelu, bt["m1b"],
                    blocks=range(2, 8))
            h2b = ipool.tile([P, KC, T], BF16, tag="tA", name="h2b")
            ln_norm(h2b, ffa, mub2, rsb2, bt["n2g"], bt["n2b"])
            out_proj(ys[2], "genp", h2b, KC, bt["genb"], wsp)

            # ---- expert 3 up-proj second half + down-projection ----------
            up_proj(h1, "m1p", xt3, KC, ACT.Gelu, bt["m1b"],
                    blocks=range(8, 16))
            out_proj(ys[3], "m2p", h1, KI, bt["m2b"], wbp)

    nc.compile()
    return nc


_PROGRAM = None


def _get_program():
    global _PROGRAM
    if _PROGRAM is None:
        _PROGRAM = build_moe_sparse()
    return _PROGRAM


def run_cores(nc, in_maps, trace=False, trace_cores=None):
    if trace:
        _install_ntff_shim()
    return run_bass_kernel_spmd(nc, in_maps, core_ids=list(range(len(in_maps))),
                                trace=trace, trace_cores=trace_cores)


# ---- host side ---------------------------------------------------------
def _gelu(x):
    try:
        from scipy.special import erf
        return 0.5 * x * (1.0 + erf(x / math.sqrt(2.0)))
    except ImportError:
        ve = np.vectorize(math.erf)
        return 0.5 * x * (1.0 + ve(x / math.sqrt(2.0)))


def _ln64(h, g, b, eps=1e-5):
    mu = h.mean(-1, keepdims=True)
    var = ((h - mu) ** 2).mean(-1, keepdims=True)
    return (h - mu) / np.sqrt(var + eps) * g + b


def _pack_w(w, kcc):
    """[K, M] fp64 -> [M//256, P, kcc, 256] bf16 contiguous tile blocks."""
    K, M = w.shape
    assert K == kcc * P
    r = w.reshape(kcc, P, M)
    blocks = [np.ascontiguousarray(r[:, :, b * 256:(b + 1) * 256]
                                   .transpose(1, 0, 2))
              for b in range(M // 256)]
    return np.stack(blocks, 0).astype(BF)


def _pack_b(b):
    n = b.shape[0] // P
    return np.ascontiguousarray(b.reshape(n, P).T.astype(np.float32))


def prepare(inputs):
    f64 = lambda n: np.asarray(inputs[n], np.float64)
    x = np.asarray(inputs["x"], np.float32).reshape(-1, H)

    # routing (host, fp64)
    lg = x.astype(np.float64) @ f64("router_w")
    lg += f64("router_b") + f64("load_balancer")
    sel = np.argsort(-lg, axis=1, kind="stable")[:, :2]
    ls = np.take_along_axis(lg, sel, 1)
    ew = np.exp(ls - ls.max(1, keepdims=True))
    gates = ew / ew.sum(1, keepdims=True)

    # folded weights (fp64)
    F = {}
    F["A1"] = f64("me_eq_w") @ f64("me_wv") @ f64("me_wo")
    F["a1"] = (f64("me_eq_b") @ f64("me_wv") + f64("me_bv")) @ f64("me_wo") \
        + f64("me_bo")
    W2o = f64("ce_wv") @ f64("ce_wo")
    F["A2"] = f64("ce_syn_w") + f64("ce_syn_w") @ W2o
    F["a2"] = f64("ce_syn_b") + f64("ce_syn_b") @ W2o + f64("ce_bv") \
        @ f64("ce_wo") + f64("ce_bo")

    wmap = {
        "w1p": (f64("sw_w1"), KC), "w3p": (f64("sw_w3"), KC),
        "w2p": (f64("sw_w2"), KI),
        "c1p": (F["A1"] @ f64("me_c1w"), KC),
        "c2p": (f64("me_c2w"), K2),
        "a2p": (F["A2"], KC), "f1p": (f64("ce_f1w"), KC),
        "f2p": (f64("ce_f2w"), K2), "genp": (f64("ce_gen_w"), KC),
        "m1p": (f64("ml_w1"), KC), "m2p": (f64("ml_w2"), KI),
    }
    bmap = {
        "c1b": F["a1"] @ f64("me_c1w") + f64("me_c1b"),
        "c2b": f64("me_c2b"),
        "a2b": F["a2"],
        "f1b": f64("ce_f1b"), "f2b": f64("ce_f2b"),
        "n1g": f64("ce_n1g"), "n1b": f64("ce_n1b"),
        "n2g": f64("ce_n2g"), "n2b": f64("ce_n2b"),
        "genb": f64("ce_gen_b"), "m1b": f64("ml_b1"), "m2b": f64("ml_b2"),
        "zb": np.zeros(H),
    }
    base = {n: _pack_w(w, kcc) for n, (w, kcc) in wmap.items()}
    base["ball"] = np.concatenate([_pack_b(bmap[n]) for n in BIASES], 1)

    meta = {"x": x, "gates": gates, "sel": sel, "F": F,
            "dev_idx": [], "dev_w": [], "ovf": []}
    in_maps = [dict(base) for _ in range(N_CORES)]
    for e in range(E):
        m = sel == e
        tok = np.nonzero(m.any(1))[0]
        we = np.where(m[:, 0][tok], gates[tok, 0], gates[tok, 1])
        dev, ovf = tok[:CAP], tok[CAP:]
        meta["dev_idx"].append(dev)
        meta["dev_w"].append(we[:len(dev)])
        meta["ovf"].append((ovf, we[len(dev):]))
        xfull = np.zeros((CAP, H), np.float32)
        xfull[:len(dev)] = x[dev]
        percore = xfull.reshape(N_CORES, T, H)
        for c in range(N_CORES):
            xc = percore[c].T.reshape(KC, P, T).transpose(1, 0, 2)
            in_maps[c][f"xg{e}"] = np.ascontiguousarray(xc).astype(BF)
    meta["in_maps"] = in_maps
    return meta


def _host_expert(e, xs, inputs, F):
    """Overflow tokens, fp64, replicating the reference formulas."""
    f64 = lambda n: np.asarray(inputs[n], np.float64)
    xs = xs.astype(np.float64)
    if e == 0:
        a = xs @ f64("sw_w1")
        g = a / (1.0 + np.exp(-a)) * (xs @ f64("sw_w3"))
        return g @ f64("sw_w2")
    if e == 1:
        t = xs @ F["A1"] + F["a1"]
        g = _gelu(t @ f64("me_c1w") + f64("me_c1b"))
        return g @ f64("me_c2w") + f64("me_c2b")
    if e == 2:
        t = xs @ F["A2"] + F["a2"]
        h2 = _ln64(t, f64("ce_n1g"), f64("ce_n1b"))
        ff = np.maximum(h2 @ f64("ce_f1w") + f64("ce_f1b"), 0.0) \
            @ f64("ce_f2w") + f64("ce_f2b")
        h2 = _ln64(h2 + ff, f64("ce_n2g"), f64("ce_n2b"))
        return h2 @ f64("ce_gen_w") + f64("ce_gen_b")
    a = _gelu(xs @ f64("ml_w1") + f64("ml_b1"))
    return a @ f64("ml_w2") + f64("ml_b2")


def combine(meta, results, inputs):
    out = np.zeros((B * S, H), np.float32)
    for e in range(E):
        ye = np.concatenate(
            [results[c][f"y{e}"].transpose(2, 1, 0).reshape(T, H)
             for c in range(N_CORES)], 0)
        dev, we = meta["dev_idx"][e], meta["dev_w"][e]
        out[dev] += (we[:, None] * ye[:len(dev)]).astype(np.float32)
        ovf, wo = meta["ovf"][e]
        if len(ovf):
            yh = _host_expert(e, meta["x"][ovf], inputs, meta["F"])
            out[ovf] += (wo[:, None] * yh).astype(np.float32)
    return out.reshape(B, S, H)


def kernel(**inputs):
    nc = _get_program()
    meta = prepare(inputs)
    res = run_cores(nc, meta["in_maps"])
    return combine(meta, [res.results[c] for c in range(N_CORES)], inputs)


# ---- NTFF profiling shim (axon) — used by test.py only ----------------
def _install_ntff_shim():
    import contextlib
    import ctypes
    import sys
    import types

    if "antenv.axon_hooks" in sys.modules:
        return
    lib = ctypes.CDLL("/opt/axon/libaxon_pjrt.so")
    if not hasattr(lib, "axon_start_nrt_profile"):
        return
    lib.axon_start_nrt_profile.argtypes = [ctypes.POINTER(ctypes.c_int64),
                                           ctypes.c_size_t]
    lib.axon_start_nrt_profile.restype = ctypes.c_int64
    lib.axon_stop_nrt_profile.argtypes = [ctypes.c_char_p]
    lib.axon_stop_nrt_profile.restype = ctypes.c_int64

    @contextlib.contextmanager
    def _hook(output_dir, device_ids):
        import jax
        jax.devices()
        if device_ids:
            ids = (ctypes.c_int64 * len(device_ids))(*device_ids)
            rc = lib.axon_start_nrt_profile(ids, len(device_ids))
        else:
            rc = lib.axon_start_nrt_profile(None, 0)
        if rc != 0:
            raise RuntimeError(f"axon_start_nrt_profile rc={rc}")
        try:
            yield
        finally:
            n = lib.axon_stop_nrt_profile(str(output_dir).encode())
            print(f"profile: {n} file(s) written to {output_dir}",
                  file=sys.stderr)

    import antenv
    mod = types.ModuleType("antenv.axon_hooks")
    mod.get_axon_ntff_profile_hook = lambda: _hook
    mod.set_axon_ntff_profile_hook = lambda hk: None
    sys.modules["antenv.axon_hooks"] = mod
    antenv.axon_hooks = mod
